# revision 1
# baseline (speedup 1.0000x reference)
"""Trainium2 Bass kernel for nn_LstmCloseModel (closed-loop LSTM over basins).

Data-parallel over the ngrid axis: 8 NeuronCores x 375 grid cells each,
replicated weights, full 365-step recurrence on-device per core.

Layout: feature-on-partition, grid-on-free.  State hT/cT live as [128,2,GP]
(H=256 split in two 128-partition chunks); gates are computed transposed
([4H, ngrid]) by PE matmuls with static weight tiles as the stationary
operand.  Matmul operands are bf16 (rel err ~4.5e-3, well under the
2e-2 budget) which halves LDWEIGHTS time vs f32r; the element-wise
cell path stays f32 on DVE (bf16 Act/DVE outputs hung the device).

Per-step schedule keeps the PE issue stream dense (HAM re-throttles the PE
clock to 1.2 GHz after a ~3.4us idle window, and a cold window makes the
next ~3.4us of matmuls run at half speed):
  pred(2) -> whh k0 x6 -> whh k1 w0 x2 -> wy1(2) -> whh k1 w1/w2 x4 ->
  wih waves (wave-3 whh interleaved after wave 0) -> win for t+1 ->
  bridge matmuls that carry the PE through the o-act/h tail into the
  next step's pred.
Gate wave order is [f, g, i, o]: sig(f)*c_prev runs early (right after
wave 0) so the DVE tail after the last gate act is just tmp -> add ->
tanh -> hmul (the tail is DVE-throughput-bound).
The y-feedback fill is ONE DVE copy_predicated (pred, incl b_out via a
ones-row matmul, into the missing lanes of the pre-staged mask*y row).  The Pool
engine is not used for bulk elementwise (measured ~2x slower than DVE
and it contends with DVE on shared tiles).  All t+1 input DMAs issue at
the head of step t so nothing waits on HBM.
"""

import os
import sys
import types

sys.path.insert(0, "/opt/trn_rl_repo")

# NTFF profile hook (timing): the image's antenv package lacks axon_hooks;
# inject an equivalent so run_bass_kernel_spmd(trace=True) can measure HW time.
try:
    import antenv

    if not hasattr(antenv, "axon_hooks"):
        from trn_agent_boot.trn_boot import _ntff_profile_via_ctypes

        _hook = _ntff_profile_via_ctypes("/opt/axon/libaxon_pjrt.so")
        _mod = types.ModuleType("antenv.axon_hooks")
        _mod.get_axon_ntff_profile_hook = lambda: _hook
        _mod.set_axon_ntff_profile_hook = lambda h: None
        sys.modules["antenv.axon_hooks"] = _mod
        antenv.axon_hooks = _mod
except Exception:
    pass

import numpy as np

import concourse.bacc as bacc
import concourse.mybir as mybir
import concourse.tile as tile
from concourse.bass import ts
from concourse.bass_utils import run_bass_kernel_spmd

NT, NGRID, NX = 365, 3000, 20
H, NY = 256, 1
NCORES = 8
G = NGRID // NCORES       # 375 grid cells per core
GP = G + (G % 2)          # padded even
KXF = NX + 2              # x-feature rows (20 real + 2 zero pads)
KY = 32                   # partition where the y-feedback row lives
F32 = mybir.dt.float32
BF16 = mybir.dt.bfloat16

N_BRIDGE = int(os.environ.get("LSTM_BRIDGE", "4"))

LAST_EXEC_NS = None
LAST_RESULTS = None


def build_nc():
    nc = bacc.Bacc("TRN2")

    xT_d = nc.declare_dram_parameter("xT", [NT, KXF, GP], BF16, isOutput=False)
    im_d = nc.declare_dram_parameter("im", [NT, GP], mybir.dt.uint8, isOutput=False)  # 1=missing
    br_d = nc.declare_dram_parameter("br", [NT, GP], BF16, isOutput=False)  # mask*y rows
    ones_d = nc.declare_dram_parameter("ones", [1, GP], BF16, isOutput=False)
    bw_d = nc.declare_dram_parameter("bw", [1, 1], BF16, isOutput=False)    # b_out
    wih_d = nc.declare_dram_parameter("wihT", [128, 2, 4 * H], BF16, isOutput=False)
    whh_d = nc.declare_dram_parameter("whhT", [128, 2, 4 * H], BF16, isOutput=False)
    win_d = nc.declare_dram_parameter("winT", [KXF, H], BF16, isOutput=False)
    wy1_d = nc.declare_dram_parameter("wy1T", [1, H], BF16, isOutput=False)
    wout_d = nc.declare_dram_parameter("woutT", [128, 2], BF16, isOutput=False)
    bg_d = nc.declare_dram_parameter("bg", [128, 8], F32, isOutput=False)
    bin_d = nc.declare_dram_parameter("bin", [128, 2], F32, isOutput=False)
    out_d = nc.declare_dram_parameter("outy", [NT, GP], F32, isOutput=True)

    AF = mybir.ActivationFunctionType
    OP = mybir.AluOpType

    with tile.TileContext(nc) as tc:
        with (
            tc.tile_pool(name="singles", bufs=1) as singles,
            tc.tile_pool(name="state", bufs=3) as state,
            tc.tile_pool(name="acts", bufs=3) as acts,
            tc.tile_pool(name="xio", bufs=3) as xio,
            tc.tile_pool(name="ps_x0", bufs=1, space="PSUM") as ps_x0,
            tc.tile_pool(name="ps_g", bufs=6, space="PSUM") as ps_g,
        ):
            # --- constants ---
            wih_s = singles.tile([128, 2, 4 * H], BF16)
            whh_s = singles.tile([128, 2, 4 * H], BF16)
            win_s = singles.tile([KXF, H], BF16)
            wy1_s = singles.tile([KY + 1, H], BF16)
            wout_s = singles.tile([128, 2], BF16)
            ones_s = singles.tile([1, GP], BF16)
            bw_s = singles.tile([1, 1], BF16)
            bg_s = singles.tile([128, 8], F32)
            bin_s = singles.tile([128, 2], F32)
            nc.sync.dma_start(out=wih_s[:], in_=wih_d[:])
            nc.sync.dma_start(out=whh_s[:], in_=whh_d[:])
            nc.sync.dma_start(out=win_s[:], in_=win_d[:])
            nc.sync.dma_start(out=wy1_s[KY : KY + 1, :], in_=wy1_d[:])
            nc.sync.dma_start(out=wout_s[:], in_=wout_d[:])
            nc.sync.dma_start(out=ones_s[:], in_=ones_d[:])
            nc.sync.dma_start(out=bw_s[:], in_=bw_d[:])
            nc.sync.dma_start(out=bg_s[:], in_=bg_d[:])
            nc.sync.dma_start(out=bin_s[:], in_=bin_d[:])

            h_prev = None  # zero at t=0; h/c terms skipped then
            c_prev = None

            # step-0 input staging
            xcat_cur = xio.tile([KY + 1, GP], BF16, tag="xcat", name="xcat0")
            nc.sync.dma_start(out=xcat_cur[0:KXF, :], in_=xT_d[0])
            nc.sync.dma_start(out=xcat_cur[KY : KY + 1, :], in_=br_d[0:1, :])
            imrow_cur = None
            x0_ps = None

            for t in range(NT):
                xcat = xcat_cur
                imrow = imrow_cur

                # ---- prefetch step-t+1 inputs (DMAs complete while t runs)
                if t + 1 < NT:
                    xcat_cur = xio.tile([KY + 1, GP], BF16, tag="xcat",
                                        name=f"xcat{t + 1}")
                    nc.sync.dma_start(out=xcat_cur[0:KXF, :], in_=xT_d[t + 1])
                    nc.sync.dma_start(out=xcat_cur[KY : KY + 1, :],
                                      in_=br_d[t + 1 : t + 2, :])
                    imrow_cur = xio.tile([1, GP], mybir.dt.uint8, tag="imrow")
                    nc.sync.dma_start(out=imrow_cur[:], in_=im_d[t + 1 : t + 2, :])

                # ---- PE stream head: pred, whh k0, then wy1 slotted into
                # the whh-k1 stream right when the y-feedback row is ready
                if t > 0:
                    with nc.named_scope("pred"):
                        yo_ps = ps_g.tile([1, GP], F32, tag="gates", name="yo_ps")
                        nc.tensor.matmul(
                            yo_ps[:], wout_s[:, 0:1], h_prev[:, 0, :],
                            start=True, stop=False,
                        )
                        nc.tensor.matmul(
                            yo_ps[:], wout_s[:, 1:2], h_prev[:, 1, :],
                            start=False, stop=False,
                        )
                        nc.tensor.matmul(
                            yo_ps[:], bw_s[0:1, 0:1], ones_s[0:1, :],
                            start=False, stop=True,
                        )

                g_pss = [
                    [ps_g.tile([128, 512], F32, tag="gates", name=f"gps{w}{jb}")
                     for jb in range(2)]
                    for w in range(3)
                ]

                def whh_mm(w, jb, k):
                    nc.tensor.matmul(
                        g_pss[w][jb][:, 0:GP],
                        whh_s[:, k, ts(2 * w + jb, 128)],
                        h_prev[:, k, :],
                        start=(k == 0), stop=False,
                    )

                if t > 0:
                    with nc.named_scope("whh"):
                        for w in range(3):
                            for jb in range(2):
                                whh_mm(w, jb, 0)
                        whh_mm(0, 0, 1)
                        whh_mm(0, 1, 1)

                # ---- fill chain on DVE: q = yo*invmk ; xcat_y = q + B (bf16)
                if t > 0:
                    with nc.named_scope("fill"):
                        nc.vector.copy_predicated(
                            xcat[KY : KY + 1, :], imrow[:], yo_ps[:]
                        )
                        # pred (incl b_out) also goes to DRAM
                        pred_sb = xio.tile([1, GP], F32, tag="pred_sb")
                        nc.scalar.copy(pred_sb[:], yo_ps[:])
                        nc.sync.dma_start(out=out_d[t - 1 : t, :], in_=pred_sb[:])

                # ---- x0: win part already accumulated (tail of t-1, or head
                # at t=0); add wy * yfill (K=1) and relu on DVE (not Act)
                if t == 0:
                    x0_ps = ps_x0.tile([128, 2, 512], F32, tag="x0ps", name="x0_ps")
                    with nc.named_scope("x0"):
                        for jb in range(2):
                            nc.tensor.matmul(
                                x0_ps[:, jb, 0:GP], win_s[:, ts(jb, 128)],
                                xcat[0:KXF, :], start=True, stop=False,
                            )
                with nc.named_scope("x0"):
                    x0_sb = acts.tile([128, 2, GP], BF16, tag="x0")
                    for jb in range(2):
                        nc.tensor.matmul(
                            x0_ps[:, jb, 0:GP], wy1_s[KY : KY + 1, ts(jb, 128)],
                            xcat[KY : KY + 1, :], start=False, stop=True,
                        )
                if t > 0:
                    with nc.named_scope("whh"):
                        for w in range(1, 3):
                            whh_mm(w, 0, 1)
                            whh_mm(w, 1, 1)
                with nc.named_scope("x0"):
                    for jb in range(2):
                        nc.vector.tensor_scalar(
                            out=x0_sb[:, jb, :], in0=x0_ps[:, jb, 0:GP],
                            scalar1=bin_s[:, jb : jb + 1], scalar2=0.0,
                            op0=OP.add, op1=OP.max,
                        )

                # ---- gate waves: w0=g(tanh), w1=i, w2=f on wih; wave3 (o)
                # whh interleaved after wave 0 so its psum allocs land on
                # freed banks (yo, wave-0) without stalling the PE.
                c_new = state.tile([128, 2, GP], F32, tag="c")
                h_new = state.tile([128, 2, GP], BF16, tag="h")
                tc_t = acts.tile([128, 2, GP], F32, tag="tanh_c")
                tmp = acts.tile([128, 2, GP], F32, tag="tmp")
                gact = []
                g3_ps = [None, None]

                def wave3_whh(jb):
                    col = ts(6 + jb, 128)
                    g3_ps[jb] = ps_g.tile([128, 512], F32, tag="gates",
                                          name=f"gps3{jb}")
                    if t > 0:
                        for k in range(2):
                            nc.tensor.matmul(
                                g3_ps[jb][:, 0:GP], whh_s[:, k, col],
                                h_prev[:, k, :], start=(k == 0), stop=False,
                            )

                for w in range(3):
                    with nc.named_scope(f"wave{w}"):
                        a_sb = acts.tile([128, 2, GP], F32, tag=f"act{w}")
                        for jb in range(2):
                            col = ts(2 * w + jb, 128)
                            for k in range(2):
                                nc.tensor.matmul(
                                    g_pss[w][jb][:, 0:GP], wih_s[:, k, col],
                                    x0_sb[:, k, :],
                                    start=(t == 0 and k == 0),
                                    stop=(k == 1),
                                )
                            nc.scalar.activation(
                                out=a_sb[:, jb, :], in_=g_pss[w][jb][:, 0:GP],
                                func=AF.Tanh if w == 1 else AF.Sigmoid,
                                bias=bg_s[:, 2 * w + jb : 2 * w + jb + 1],
                            )
                        gact.append(a_sb)
                    if w == 0:
                        if t > 0:
                            with nc.named_scope("cell"):
                                # c_f = sig(f)*c_prev early, off the tail path
                                for jb in range(2):
                                    nc.vector.tensor_mul(
                                        c_new[:, jb, :],
                                        gact[0][:, jb, :], c_prev[:, jb, :],
                                    )
                    if w == 2:
                        with nc.named_scope("cell"):
                            # tmp = tanh(g)*sig(i); c += tmp; tanh(c).
                            # j0 chain first so h j0 lands asap.
                            for jb in range(2):
                                nc.vector.tensor_mul(
                                    tmp[:, jb, :],
                                    gact[1][:, jb, :], gact[2][:, jb, :],
                                )
                                if t > 0:
                                    nc.vector.tensor_add(
                                        c_new[:, jb, :],
                                        c_new[:, jb, :], tmp[:, jb, :],
                                    )
                                else:
                                    nc.vector.tensor_copy(
                                        c_new[:, jb, :], tmp[:, jb, :]
                                    )
                                nc.scalar.activation(
                                    out=tc_t[:, jb, :], in_=c_new[:, jb, :],
                                    func=AF.Tanh,
                                )

                # ---- wave3 wih + o act + h per H-half
                with nc.named_scope("wave3"):
                    wave3_whh(0)
                    wave3_whh(1)
                    so = acts.tile([128, 2, GP], F32, tag="act3")
                    for jb in range(2):
                        col = ts(6 + jb, 128)
                        for k in range(2):
                            nc.tensor.matmul(
                                g3_ps[jb][:, 0:GP], wih_s[:, k, col],
                                x0_sb[:, k, :],
                                start=(t == 0 and k == 0), stop=(k == 1),
                            )
                        nc.scalar.activation(
                            out=so[:, jb, :], in_=g3_ps[jb][:, 0:GP],
                            func=AF.Sigmoid,
                            bias=bg_s[:, 6 + jb : 6 + jb + 1],
                        )
                        nc.vector.tensor_mul(
                            h_new[:, jb, :], so[:, jb, :], tc_t[:, jb, :]
                        )

                # ---- tail: win matmuls for t+1, then bridge matmuls that
                # carry the PE through the o-act/hmul chain into t+1's pred.
                if t + 1 < NT:
                    x0_ps = ps_x0.tile([128, 2, 512], F32, tag="x0ps",
                                       name="x0_ps")
                    with nc.named_scope("x0"):
                        for jb in range(2):
                            nc.tensor.matmul(
                                x0_ps[:, jb, 0:GP], win_s[:, ts(jb, 128)],
                                xcat_cur[0:KXF, :], start=True, stop=False,
                            )
                with nc.named_scope("warm"):
                    dmy = ps_g.tile([128, 512], F32, tag="gates", name="dmy")
                    for d in range(N_BRIDGE):
                        nc.tensor.matmul(
                            dmy[:, 0:GP], whh_s[:, 0, ts(d, 128)],
                            x0_sb[:, 0, :], start=True, stop=True,
                        )

                h_prev, c_prev = h_new, c_new

            # final output row from h_{NT-1}
            with nc.named_scope("pred"):
                yo_ps = ps_g.tile([1, GP], F32, tag="gates", name="yo_ps")
                nc.tensor.matmul(
                    yo_ps[:], wout_s[:, 0:1], h_prev[:, 0, :],
                    start=True, stop=False,
                )
                nc.tensor.matmul(
                    yo_ps[:], wout_s[:, 1:2], h_prev[:, 1, :],
                    start=False, stop=False,
                )
                nc.tensor.matmul(
                    yo_ps[:], bw_s[0:1, 0:1], ones_s[0:1, :],
                    start=False, stop=True,
                )
                pred_sb = xio.tile([1, GP], F32, tag="pred_sb")
                nc.vector.tensor_copy(pred_sb[:], yo_ps[:])
                nc.sync.dma_start(out=out_d[NT - 1 : NT, :], in_=pred_sb[:])

    nc.finalize()
    return nc


def kernel(x, y, w_in, b_in, w_ih, b_ih, w_hh, b_hh, w_out, b_out):
    global LAST_EXEC_NS, LAST_RESULTS
    import ml_dtypes

    bf16 = ml_dtypes.bfloat16
    x = np.asarray(x, np.float32)
    y = np.asarray(y, np.float32)

    # gate reorder [i,f,g,o] -> wave order [f,g,i,o]
    perm = np.concatenate(
        [np.arange(H, 2 * H), np.arange(2 * H, 3 * H), np.arange(0, H),
         np.arange(3 * H, 4 * H)]
    )
    wih_r = np.asarray(w_ih, np.float32)[perm]          # [1024, 256]
    whh_r = np.asarray(w_hh, np.float32)[perm]
    bg_r = (np.asarray(b_ih, np.float32) + np.asarray(b_hh, np.float32))[perm]

    wih_dev = np.ascontiguousarray(
        wih_r.T.reshape(2, 128, 4 * H).transpose(1, 0, 2)).astype(bf16)
    whh_dev = np.ascontiguousarray(
        whh_r.T.reshape(2, 128, 4 * H).transpose(1, 0, 2)).astype(bf16)
    bg_dev = np.ascontiguousarray(bg_r.reshape(8, 128).T)   # [128,8]

    # winT covers xcat rows 0..21 (20 x features + 2 zero rows);
    # wy1T is the y-feedback weight column (xcat row KY)
    w_in = np.asarray(w_in, np.float32)                      # [256, 21]
    win_re = np.concatenate(
        [w_in[:, :NX], np.zeros((H, 2), np.float32)], axis=1)  # [256, 22]
    win_dev = np.ascontiguousarray(win_re.T).astype(bf16)    # [22, 256]
    wy1_dev = np.ascontiguousarray(
        w_in[:, NX : NX + 1].T).astype(bf16)                 # [1, 256]
    bin_dev = np.ascontiguousarray(
        np.asarray(b_in, np.float32).reshape(2, 128).T)      # [128,2]

    wout_dev = np.ascontiguousarray(
        np.asarray(w_out, np.float32).reshape(2, 128).T).astype(bf16)  # [128,2]
    bout_f = float(np.asarray(b_out).reshape(-1)[0])

    y2 = y[:, :, 0]                                          # [NT, NGRID]
    mk_full = (~np.isnan(y2)).astype(np.float32)             # 1 where observed
    y0_full = np.nan_to_num(y2, nan=0.0).astype(np.float32)
    im_full = (1.0 - mk_full).astype(np.uint8)               # 1 where missing
    br_full = mk_full * y0_full                              # [NT, NGRID]

    nc = build_nc()
    in_maps = []
    for c in range(NCORES):
        g0, g1 = c * G, (c + 1) * G
        xT = np.zeros((NT, KXF, GP), np.float32)
        xT[:, :NX, :G] = x[:, g0:g1, :].transpose(0, 2, 1)
        im = np.zeros((NT, GP), np.uint8)
        im[:, :G] = im_full[:, g0:g1]
        br = np.zeros((NT, GP), np.float32)
        br[:, :G] = br_full[:, g0:g1]
        in_maps.append(
            {
                "xT": xT.astype(bf16), "br": br.astype(bf16),
                "im": im,
                "ones": np.ones((1, GP), np.float32).astype(bf16),
                "bw": np.full((1, 1), bout_f, np.float32).astype(bf16),
                "wihT": wih_dev, "whhT": whh_dev, "winT": win_dev,
                "wy1T": wy1_dev, "woutT": wout_dev,
                "bg": bg_dev, "bin": bin_dev,
            }
        )

    # transient INTERNAL/profiler failures have been observed on known-good
    # binaries; retry with a cool-down so a flaky device state can clear
    res = None
    for attempt in range(5):
        try:
            res = run_bass_kernel_spmd(nc, in_maps, core_ids=list(range(NCORES)))
            break
        except Exception:
            if attempt == 4:
                raise
            import time
            time.sleep(15)
    LAST_EXEC_NS = res.exec_time_ns
    LAST_RESULTS = res

    out = np.empty((NT, NGRID, NY), np.float32)
    for c in range(NCORES):
        out[:, c * G : (c + 1) * G, 0] = res.results[c]["outy"][:, :G]
    return out

